# revision 18
# baseline (speedup 1.0000x reference)
"""Trainium2 Bass kernel for a 2-layer GENConv (softmax aggr) + LayerNorm GNN block.

Distribution: graph-partitioned across 8 NeuronCores via a Fiedler (spectral 1D)
node ordering. The per-channel softmax aggregation collapses to two SpMMs
because GENConv softmax logits depend only on the source node:

  r = relu(x); w = exp(t*r); q = w*r
  num = A @ q;  den = A @ w;  agg = num/den + eps     (exactly the reference
  softmax aggregation; the max-shift and the eps inside exp cancel)

Each core owns 4 contiguous dst blocks of 128 nodes and receives a GATHERED
source window: its 4 own tiles first, then the sorted distinct out-of-own
source rows (halo), zero-padded to a uniform TU tiles (SPMD uniformity lives
in the data, not the code). The A^T slabs are fp8 (0/1 counts, exact) and the
q/w operands are fp8, so the SpMM runs in DoubleRow (double-pumped) mode.
Block epilogues are software-pipelined behind the next block's SpMM so the
tensor engine never waits on the vector division chain.

Three SPMD launches: conv1 -> (host regathers x1) -> conv2 + LN + per-core
transposed column sums -> (host stacks partials) -> tiny finalize matvec.
The host does no float arithmetic: only slicing/permutation/concatenation.
"""

import ml_dtypes
import numpy as np

import concourse.bass as bass
import concourse.bacc as bacc
import concourse.mybir as mybir
import concourse.tile as tile
import concourse.masks as masks
from concourse.bass_utils import run_bass_kernel_spmd

F32 = mybir.dt.float32
BF16 = mybir.dt.bfloat16
F8 = mybir.dt.float8e4
AF = mybir.ActivationFunctionType
ALU = mybir.AluOpType
DR = mybir.MatmulPerfMode.DoubleRow

N_CORES = 8
H = 768
CHT = H // 128           # channel tiles = 6
EPS_MSG = 1e-7
LN_EPS = 1e-5

_cache = {}


# ----------------------------------------------------------------------------
# Host-side graph preprocessing (index work only — no float math on data).
# ----------------------------------------------------------------------------

def _ordering(src, dst, n):
    """1D spectral (Fiedler) layout of the graph; falls back to RCM/identity."""
    import scipy.sparse as sp
    a = sp.csr_matrix(
        (np.ones(len(src), dtype=np.float64), (dst, src)), shape=(n, n)
    )
    asym = ((a + a.T) > 0).astype(np.float64)
    try:
        from scipy.sparse.linalg import eigsh
        lap = sp.diags(np.asarray(asym.sum(1)).ravel()) - asym
        _, vecs = eigsh(lap, k=2, sigma=-1e-4, which="LM")
        return np.argsort(vecs[:, 1]).astype(np.int64)
    except Exception:
        try:
            from scipy.sparse.csgraph import reverse_cuthill_mckee
            return np.asarray(
                reverse_cuthill_mckee(asym.tocsr(), symmetric_mode=True)
            ).astype(np.int64)
        except Exception:
            return np.arange(n, dtype=np.int64)


def _prepare(edge_index, n):
    src = np.asarray(edge_index[0], dtype=np.int64)
    dst = np.asarray(edge_index[1], dtype=np.int64)
    perm = _ordering(src, dst, n)           # new position i holds old node perm[i]
    inv = np.empty(n, dtype=np.int64)
    inv[perm] = np.arange(n)
    ns, nd = inv[src], inv[dst]             # edges in new coordinates

    nb = n // 128
    bpc = nb // N_CORES                     # dst blocks per core (4)

    # per-core gathered source lists: [own rows | sorted halo rows | -1 pad]
    halos = []
    for c in range(N_CORES):
        lo, hi = c * bpc * 128, (c + 1) * bpc * 128
        m = (nd >= lo) & (nd < hi)
        srcs = np.unique(ns[m])
        halos.append(srcs[(srcs < lo) | (srcs >= hi)])
    tu = bpc + max((len(h) + 127) // 128 for h in halos)   # uniform tiles
    tu += tu % 2                                           # even (DoubleRow pairs)
    glists, abands = [], []
    for c in range(N_CORES):
        lo, hi = c * bpc * 128, (c + 1) * bpc * 128
        glist = np.full(tu * 128, -1, dtype=np.int64)
        nown = hi - lo
        glist[:nown] = np.arange(lo, hi)
        glist[nown:nown + len(halos[c])] = halos[c]
        glists.append(glist)
        pos = np.full(n, -1, dtype=np.int64)
        valid = glist >= 0
        pos[glist[valid]] = np.arange(tu * 128)[valid]
        m = (nd >= lo) & (nd < hi)
        p = pos[ns[m]]
        d = nd[m] - lo
        assert (p >= 0).all()
        ab = np.zeros((128, bpc * tu * 128), dtype=np.float32)
        np.add.at(ab, (p % 128, ((d // 128) * tu + p // 128) * 128 + d % 128), 1.0)
        abands.append(ab.astype(ml_dtypes.float8_e4m3fn))

    return dict(perm=perm, inv=inv, tu=tu, bpc=bpc, glists=glists,
                abands=abands)


def _gather_rows(full, glist):
    """full[glist] with -1 -> zero row; returns fp8e4m3."""
    out = np.zeros((len(glist), full.shape[1]), dtype=ml_dtypes.float8_e4m3fn)
    sel = glist >= 0
    out[sel] = full[glist[sel]].astype(ml_dtypes.float8_e4m3fn)
    return out


def _pack_wt(w, dtype=ml_dtypes.bfloat16):
    """[Hout, Hin] weight -> partition-major packed W.T tiles [128, (Hin/128)*Hout]:
    out[p, c*Hout + o] = W[o, c*128 + p]"""
    h_out, h_in = w.shape
    nt = h_in // 128
    out = np.empty((128, nt * h_out), dtype=np.float32)
    for c in range(nt):
        out[:, c * h_out:(c + 1) * h_out] = w[:, c * 128:(c + 1) * 128].T
    return np.ascontiguousarray(out.astype(dtype))


# ----------------------------------------------------------------------------
# Bass programs.
# ----------------------------------------------------------------------------

def _build_conv(prep, layer2):
    """One GENConv layer over the gathered source window.

    layer2=False: out xout [bpc*128, H] bf16 (new node features x1).
    layer2=True : DeepGCN tail (LayerNorm+relu+residual) and per-core
    transposed column sums csout [128, 2*CHT] f32
    (csout[:, j] = colsum of channel tile j; first CHT cols x1, last CHT x2).
    """
    tu, bpc = prep["tu"], prep["bpc"]
    nc = bacc.Bacc("TRN2", target_bir_lowering=False, debug=False,
                   enable_asserts=False, num_devices=N_CORES)
    xg = nc.dram_tensor("xg", [tu * 128, H], F8, kind="ExternalInput")
    ab = nc.dram_tensor("ab", [128, bpc * tu * 128], F8, kind="ExternalInput")
    wt = nc.dram_tensor("wt", [128, CHT * H], BF16, kind="ExternalInput")
    br = nc.dram_tensor("br", [128, H], F32, kind="ExternalInput")
    ts = nc.dram_tensor("ts", [128, 1], F32, kind="ExternalInput")
    if layer2:
        lngr = nc.dram_tensor("lngr", [128, H], F32, kind="ExternalInput")
        lnbr = nc.dram_tensor("lnbr", [128, H], F32, kind="ExternalInput")
        csout = nc.dram_tensor("csout", [128, 2 * CHT], F32, kind="ExternalOutput")
    else:
        xout = nc.dram_tensor("xout", [bpc * 128, H], F8, kind="ExternalOutput")
        xout_r = xout.rearrange("(n p) d -> n p d", p=128)

    xg_r = xg.rearrange("(n p) d -> n p d", p=128)

    with tile.TileContext(nc) as tc:
        with (
            tc.tile_pool(name="persist", bufs=1) as pp,
            tc.tile_pool(name="epi", bufs=2) as ep,
        ):
            # tiny params first (exp scale needed by the qw pass)
            ts_sb = pp.tile([128, 1], F32)
            nc.sync.dma_start(ts_sb[:], ts[:])
            eps_sb = pp.tile([128, 1], F32)
            nc.gpsimd.memset(eps_sb[:], EPS_MSG)
            # w is scaled by 1/64 inside the exp (softmax ratio is invariant)
            # so fp8e4m3 never overflows: exp(t*r) <= 448*64 is covered.
            lge_sb = pp.tile([128, 1], F32)
            nc.gpsimd.memset(lge_sb[:], float(-np.log(64.0)))

            # A slabs first: the first SpMM needs slab 0 + two qw tiles only
            ab_sb = pp.tile([128, bpc * tu * 128], F8)
            for bl in range(bpc):
                nc.sync.dma_start(ab_sb[:, bl * tu * 128:(bl + 1) * tu * 128],
                                  ab[:, bl * tu * 128:(bl + 1) * tu * 128])
            ab_r = ab_sb[:].rearrange("p (b t k m) -> p b t k m", b=bpc, k=2, m=128)

            # source pass: r = relu(x); w8 = exp(t*r); q8 = r*w8  (fp8 operands)
            xg_sb = pp.tile([128, tu * H], F8)
            qw8 = pp.tile([128, tu * 2 * H], F8)
            qw8_r = qw8[:].rearrange("p (t f) -> p t f", f=2 * H)
            for s in range(tu):
                xs = xg_sb[:, s * H:(s + 1) * H]
                nc.sync.dma_start(xs, xg_r[s])
                q8 = qw8[:, 2 * s * H:(2 * s + 1) * H]
                w8 = qw8[:, (2 * s + 1) * H:(2 * s + 2) * H]
                rs = ep.tile([128, H], BF16, tag="rs")
                ws = ep.tile([128, H], BF16, tag="ws")
                nc.vector.tensor_scalar_max(rs[:], xs, 0.0)
                nc.scalar.activation(ws[:], rs[:], AF.Exp,
                                     bias=lge_sb[:, 0:1], scale=ts_sb[:, 0:1])
                nc.vector.tensor_mul(q8, rs[:], ws[:])
                nc.scalar.copy(w8, ws[:])

            wt_sb = pp.tile([128, CHT * H], BF16)
            nc.sync.dma_start(wt_sb[:], wt[:])
            br_sb = pp.tile([128, H], F32)
            nc.sync.dma_start(br_sb[:], br[:])
            ident = pp.tile([128, 128], F32)
            masks.make_identity(nc, ident[:])
            if layer2:
                lng_sb = pp.tile([128, H], F32)
                lnb_sb = pp.tile([128, H], F32)
                nc.sync.dma_start(lng_sb[:], lngr[:])
                nc.sync.dma_start(lnb_sb[:], lnbr[:])
                lneps_sb = pp.tile([128, 1], F32)
                nc.gpsimd.memset(lneps_sb[:], LN_EPS)
                ones_sb = pp.tile([128, 1], BF16)
                nc.gpsimd.memset(ones_sb[:], 1.0)
                acc1 = pp.tile([128, H], F32)
                acc2 = pp.tile([128, H], F32)
                nc.gpsimd.memset(acc1[:], 0.0)
                nc.gpsimd.memset(acc2[:], 0.0)

            with (
                tc.tile_pool(name="psA", bufs=2, space="PSUM") as psA,
                tc.tile_pool(name="psE", bufs=2, space="PSUM") as psE,
            ):
                aggs = [None] * bpc

                def spmm(bl):
                    agg = psA.tile([128, 2 * H], F32, tag="agg")
                    aggs[bl] = agg
                    for td in range(tu // 2):
                        at2 = ab_r[:, bl, td]
                        for ch in range(3):
                            nc.tensor.matmul(
                                agg[:, ch * 512:(ch + 1) * 512],
                                at2,
                                qw8_r[:, 2 * td:2 * td + 2, ch * 512:(ch + 1) * 512],
                                start=(td == 0), stop=(td == tu // 2 - 1),
                                perf_mode=DR,
                            )

                def epilogue(bl):
                    agg = aggs[bl]
                    xo = xg_sb[:, bl * H:(bl + 1) * H]      # own x tile (bf16)
                    m = ep.tile([128, H], F32, tag="m")
                    rec = ep.tile([128, H], F32, tag="rec")
                    nc.vector.reciprocal_approx_fast(rec[:], agg[:, H:2 * H])
                    nc.vector.tensor_mul(m[:], agg[:, 0:H], rec[:])
                    nc.vector.tensor_add(m[:], m[:], xo)
                    # transpose m -> lhsT tiles, then x_new = m @ W.T + b
                    mt = ep.tile([128, H], BF16, tag="mt")
                    for c in range(CHT):
                        tp = psE.tile([128, 128], F32, tag="e")
                        nc.tensor.transpose(tp[:], m[:, c * 128:(c + 1) * 128], ident[:])
                        nc.scalar.copy(mt[:, c * 128:(c + 1) * 128], tp[:])
                    xps = psA.tile([128, H], F32, tag="agg")
                    for c in range(CHT):
                        nc.tensor.matmul(
                            xps[:, 0:512], mt[:, c * 128:(c + 1) * 128],
                            wt_sb[:, c * H:c * H + 512],
                            start=(c == 0), stop=(c == CHT - 1))
                        nc.tensor.matmul(
                            xps[:, 512:H], mt[:, c * 128:(c + 1) * 128],
                            wt_sb[:, c * H + 512:(c + 1) * H],
                            start=(c == 0), stop=(c == CHT - 1))
                    if not layer2:
                        xnb = ep.tile([128, H], F8, tag="xnb")
                        nc.vector.tensor_add(xnb[:], xps[:], br_sb[:])
                        nc.sync.dma_start(xout_r[bl], xnb[:])
                    else:
                        xn = ep.tile([128, H], F32, tag="xn")
                        nc.vector.tensor_add(xn[:], xps[:], br_sb[:])
                        # LayerNorm over channels, relu, then x2 = hn + x1_own
                        stats = ep.tile([128, 3, 6], F32, tag="bnst")
                        xn_r = xn[:].rearrange("p (g f) -> p g f", f=256)
                        for g3 in range(3):
                            nc.vector.bn_stats(stats[:, g3, :], xn_r[:, g3])
                        mv = ep.tile([128, 2], F32, tag="mv")
                        nc.vector.bn_aggr(mv[:], stats[:])
                        rstd = ep.tile([128, 1], F32, tag="rstd")
                        veps = ep.tile([128, 1], F32, tag="veps")
                        nc.vector.tensor_scalar(veps[:], mv[:, 1:2],
                                                lneps_sb[:, 0:1], None, ALU.add)
                        nc.vector.reciprocal_approx_fast(rstd[:], veps[:])
                        nc.scalar.sqrt(rstd[:], rstd[:])
                        nmr = ep.tile([128, 1], F32, tag="nmr")
                        nc.vector.tensor_mul(nmr[:], mv[:, 0:1], rstd[:])
                        nc.vector.tensor_scalar_mul(nmr[:], nmr[:], -1.0)
                        hn = ep.tile([128, H], F32, tag="hn")
                        nc.scalar.activation(hn[:], xn[:], AF.Identity,
                                             bias=nmr[:, 0:1], scale=rstd[:, 0:1])
                        nc.gpsimd.tensor_mul(hn[:], hn[:], lng_sb[:])
                        nc.gpsimd.tensor_add(hn[:], hn[:], lnb_sb[:])
                        nc.scalar.activation(hn[:], hn[:], AF.Relu)
                        x2 = ep.tile([128, H], F32, tag="x2")
                        nc.gpsimd.tensor_add(x2[:], hn[:], xo)
                        # per-partition running sums (cross-partition reduce later)
                        nc.gpsimd.tensor_add(acc1[:], acc1[:], xo)
                        nc.gpsimd.tensor_add(acc2[:], acc2[:], x2[:])

                # software pipeline: SpMM(bl+1) issues before epilogue(bl)
                spmm(0)
                for bl in range(1, bpc):
                    spmm(bl)
                    epilogue(bl - 1)
                epilogue(bpc - 1)

            if layer2:
                with tc.tile_pool(name="psF", bufs=1, space="PSUM") as psF:
                    # transposed column sums: cs[:, j] = colsum of channel tile j
                    acc1b = pp.tile([128, H], BF16)
                    acc2b = pp.tile([128, H], BF16)
                    nc.scalar.copy(acc1b[:], acc1[:])
                    nc.scalar.copy(acc2b[:], acc2[:])
                    cs_ps = psF.tile([128, 2 * CHT], F32)
                    for j in range(CHT):
                        nc.tensor.matmul(cs_ps[:, j:j + 1],
                                         acc1b[:, j * 128:(j + 1) * 128],
                                         ones_sb[:], start=True, stop=True)
                        nc.tensor.matmul(cs_ps[:, CHT + j:CHT + j + 1],
                                         acc2b[:, j * 128:(j + 1) * 128],
                                         ones_sb[:], start=True, stop=True)
                    cs_sb = pp.tile([128, 2 * CHT], F32)
                    nc.scalar.copy(cs_sb[:], cs_ps[:])
                    nc.sync.dma_start(csout[:], cs_sb[:])
    nc.compile()
    return nc


def _build_final(n):
    """Sum per-core transposed colsum partials, matvec through Wc, + bc + x0."""
    nc = bacc.Bacc("TRN2", target_bir_lowering=False, debug=False,
                   enable_asserts=False, num_devices=N_CORES)
    parts = nc.dram_tensor("parts", [128, N_CORES * 2 * CHT], F32, kind="ExternalInput")
    wct = nc.dram_tensor("wct", [128, 2 * CHT * H], BF16, kind="ExternalInput")
    bcr = nc.dram_tensor("bcr", [1, H], F32, kind="ExternalInput")
    x0r = nc.dram_tensor("x0r", [1, H], F32, kind="ExternalInput")
    row0 = nc.dram_tensor("row0", [1, H], F32, kind="ExternalOutput")

    with tile.TileContext(nc) as tc:
        with (
            tc.tile_pool(name="sb", bufs=1) as sb,
            tc.tile_pool(name="ps", bufs=1, space="PSUM") as ps,
        ):
            wct_sb = sb.tile([128, 2 * CHT * H], BF16)
            nc.sync.dma_start(wct_sb[:], wct[:])
            pt = sb.tile([128, N_CORES * 2 * CHT], F32)
            nc.sync.dma_start(pt[:], parts[:])
            acc = sb.tile([128, 2 * CHT], F32)
            nc.vector.tensor_reduce(
                acc[:], pt[:].rearrange("p (a d) -> p d a", a=N_CORES),
                mybir.AxisListType.X, ALU.add)
            nc.vector.tensor_scalar_mul(acc[:], acc[:], 1.0 / n)
            accb = sb.tile([128, 2 * CHT], BF16)
            nc.scalar.copy(accb[:], acc[:])

            g_ps = ps.tile([1, H], F32)
            for j in range(2 * CHT):
                for lo, hi in ((0, 512), (512, H)):   # per-bank chunks
                    nc.tensor.matmul(
                        g_ps[:, lo:hi],
                        accb[:, j:j + 1],
                        wct_sb[:, j * H + lo:j * H + hi],
                        start=(j == 0), stop=(j == 2 * CHT - 1))
            bc_sb = sb.tile([1, H], F32)
            x0_sb = sb.tile([1, H], F32)
            out_sb = sb.tile([1, H], F32)
            nc.sync.dma_start(bc_sb[:], bcr[:])
            nc.sync.dma_start(x0_sb[:], x0r[:])
            nc.vector.tensor_add(out_sb[:], g_ps[:], bc_sb[:])
            nc.vector.tensor_add(out_sb[:], out_sb[:], x0_sb[:])
            nc.sync.dma_start(row0[:], out_sb[:])
    nc.compile()
    return nc


def kernel(**inputs):
    x = np.asarray(inputs["x"], dtype=np.float32)
    w1 = np.asarray(inputs["W1"], dtype=np.float32)
    b1 = np.asarray(inputs["b1"], dtype=np.float32)
    t1 = np.float32(np.asarray(inputs["t1"]))
    w2 = np.asarray(inputs["W2"], dtype=np.float32)
    b2 = np.asarray(inputs["b2"], dtype=np.float32)
    t2 = np.float32(np.asarray(inputs["t2"]))
    ln_g = np.asarray(inputs["ln_g"], dtype=np.float32)
    ln_b = np.asarray(inputs["ln_b"], dtype=np.float32)
    wc = np.asarray(inputs["Wc"], dtype=np.float32)
    bc = np.asarray(inputs["bc"], dtype=np.float32)
    ei = np.asarray(inputs["edge_index"])

    n = x.shape[1]
    ekey = (ei.shape[1], n,
            int(np.bitwise_xor.reduce(ei[0].astype(np.int64) * 31 + ei[1])))
    if ekey not in _cache:
        prep = _prepare(ei, n)
        progs = dict(
            conv=_build_conv(prep, False),
            tail=_build_conv(prep, True),
            fin=_build_final(n),
        )
        _cache[ekey] = (prep, progs)
    prep, progs = _cache[ekey]

    xp = np.ascontiguousarray(x[0][prep["perm"]])    # permuted node features
    t1r = np.full((128, 1), t1, dtype=np.float32)
    t2r = np.full((128, 1), t2, dtype=np.float32)
    w1t, w2t = _pack_wt(w1), _pack_wt(w2)
    b1r = np.ascontiguousarray(np.broadcast_to(b1, (128, H)))
    b2r = np.ascontiguousarray(np.broadcast_to(b2, (128, H)))
    lngr = np.ascontiguousarray(np.broadcast_to(ln_g, (128, H)))
    lnbr = np.ascontiguousarray(np.broadcast_to(ln_b, (128, H)))

    cores = list(range(N_CORES))

    # --- launch 1: conv1 ---
    maps1 = [dict(xg=_gather_rows(xp, prep["glists"][c]), ab=prep["abands"][c],
                  wt=w1t, br=b1r, ts=t1r) for c in cores]
    res1 = run_bass_kernel_spmd(progs["conv"], maps1, core_ids=cores)
    x1 = np.concatenate([res1.results[c]["xout"] for c in cores], axis=0)

    # --- launch 2: conv2 + LN + transposed colsums ---
    maps2 = [dict(xg=_gather_rows(x1, prep["glists"][c]), ab=prep["abands"][c],
                  wt=w2t, br=b2r, ts=t2r, lngr=lngr, lnbr=lnbr) for c in cores]
    res2 = run_bass_kernel_spmd(progs["tail"], maps2, core_ids=cores)
    parts = np.ascontiguousarray(
        np.concatenate([res2.results[c]["csout"] for c in cores], axis=1))

    # --- launch 3: finalize row 0 ---
    maps3 = [dict(parts=parts, wct=_pack_wt(wc),
                  bcr=bc.reshape(1, H).astype(np.float32),
                  x0r=np.ascontiguousarray(x[0, 0:1, :])) for _ in cores]
    res3 = run_bass_kernel_spmd(progs["fin"], maps3, core_ids=cores)
    row0 = res3.results[0]["row0"][0]

    out = x.copy()
    out[0, 0, :] = row0
    return out


# revision 20
# speedup vs baseline: 1.0407x; 1.0407x over previous
"""Trainium2 Bass kernel for a 2-layer GENConv (softmax aggr) + LayerNorm GNN block.

Distribution: graph-partitioned across 8 NeuronCores via a Fiedler (spectral 1D)
node ordering. The per-channel softmax aggregation collapses to two SpMMs
because GENConv softmax logits depend only on the source node:

  r = relu(x); w = exp(t*r); q = w*r
  num = A @ q;  den = A @ w;  agg = num/den + eps     (exactly the reference
  softmax aggregation; the max-shift and the eps inside exp cancel)

Each core owns 4 contiguous dst blocks of 128 nodes and receives a GATHERED
source window: its 4 own tiles first, then the sorted distinct out-of-own
source rows (halo), zero-padded to a uniform TU tiles (SPMD uniformity lives
in the data, not the code). The A^T slabs are fp8 (0/1 counts, exact) and the
q/w operands are fp8, so the SpMM runs in DoubleRow (double-pumped) mode.
Block epilogues are software-pipelined behind the next block's SpMM so the
tensor engine never waits on the vector division chain.

Three SPMD launches: conv1 -> (host regathers x1) -> conv2 + LN + per-core
transposed column sums -> (host stacks partials) -> tiny finalize matvec.
The host does no float arithmetic: only slicing/permutation/concatenation.
"""

import ml_dtypes
import numpy as np

import concourse.bass as bass
import concourse.bacc as bacc
import concourse.mybir as mybir
import concourse.tile as tile
import concourse.masks as masks
from concourse.bass_utils import run_bass_kernel_spmd

F32 = mybir.dt.float32
BF16 = mybir.dt.bfloat16
F8 = mybir.dt.float8e4
AF = mybir.ActivationFunctionType
ALU = mybir.AluOpType
DR = mybir.MatmulPerfMode.DoubleRow

N_CORES = 8
H = 768
CHT = H // 128           # channel tiles = 6
EPS_MSG = 1e-7
LN_EPS = 1e-5

_cache = {}


# ----------------------------------------------------------------------------
# Host-side graph preprocessing (index work only — no float math on data).
# ----------------------------------------------------------------------------

def _ordering(src, dst, n):
    """1D spectral (Fiedler) layout of the graph; falls back to RCM/identity."""
    import scipy.sparse as sp
    a = sp.csr_matrix(
        (np.ones(len(src), dtype=np.float64), (dst, src)), shape=(n, n)
    )
    asym = ((a + a.T) > 0).astype(np.float64)
    try:
        from scipy.sparse.linalg import eigsh
        lap = sp.diags(np.asarray(asym.sum(1)).ravel()) - asym
        _, vecs = eigsh(lap, k=2, sigma=-1e-4, which="LM")
        return np.argsort(vecs[:, 1]).astype(np.int64)
    except Exception:
        try:
            from scipy.sparse.csgraph import reverse_cuthill_mckee
            return np.asarray(
                reverse_cuthill_mckee(asym.tocsr(), symmetric_mode=True)
            ).astype(np.int64)
        except Exception:
            return np.arange(n, dtype=np.int64)


def _prepare(edge_index, n):
    src = np.asarray(edge_index[0], dtype=np.int64)
    dst = np.asarray(edge_index[1], dtype=np.int64)
    perm = _ordering(src, dst, n)           # new position i holds old node perm[i]
    inv = np.empty(n, dtype=np.int64)
    inv[perm] = np.arange(n)
    ns, nd = inv[src], inv[dst]             # edges in new coordinates

    nb = n // 128
    bpc = nb // N_CORES                     # dst blocks per core (4)

    # per-core gathered source lists: [own rows | sorted halo rows | -1 pad]
    halos = []
    for c in range(N_CORES):
        lo, hi = c * bpc * 128, (c + 1) * bpc * 128
        m = (nd >= lo) & (nd < hi)
        srcs = np.unique(ns[m])
        halos.append(srcs[(srcs < lo) | (srcs >= hi)])
    tu = bpc + max((len(h) + 127) // 128 for h in halos)   # uniform tiles
    tu += tu % 2                                           # even (DoubleRow pairs)
    glists, abands = [], []
    for c in range(N_CORES):
        lo, hi = c * bpc * 128, (c + 1) * bpc * 128
        glist = np.full(tu * 128, -1, dtype=np.int64)
        nown = hi - lo
        glist[:nown] = np.arange(lo, hi)
        glist[nown:nown + len(halos[c])] = halos[c]
        glists.append(glist)
        pos = np.full(n, -1, dtype=np.int64)
        valid = glist >= 0
        pos[glist[valid]] = np.arange(tu * 128)[valid]
        m = (nd >= lo) & (nd < hi)
        p = pos[ns[m]]
        d = nd[m] - lo
        assert (p >= 0).all()
        ab = np.zeros((128, bpc * tu * 128), dtype=np.float32)
        np.add.at(ab, (p % 128, ((d // 128) * tu + p // 128) * 128 + d % 128), 1.0)
        abands.append(ab.astype(ml_dtypes.float8_e4m3fn))

    return dict(perm=perm, inv=inv, tu=tu, bpc=bpc, glists=glists,
                abands=abands)


def _gather_rows(full, glist):
    """full[glist] with -1 -> zero row; returns fp8e4m3."""
    out = np.zeros((len(glist), full.shape[1]), dtype=ml_dtypes.bfloat16)
    sel = glist >= 0
    out[sel] = full[glist[sel]].astype(ml_dtypes.bfloat16)
    return out


def _pack_wt(w, dtype=ml_dtypes.bfloat16):
    """[Hout, Hin] weight -> partition-major packed W.T tiles [128, (Hin/128)*Hout]:
    out[p, c*Hout + o] = W[o, c*128 + p]"""
    h_out, h_in = w.shape
    nt = h_in // 128
    out = np.empty((128, nt * h_out), dtype=np.float32)
    for c in range(nt):
        out[:, c * h_out:(c + 1) * h_out] = w[:, c * 128:(c + 1) * 128].T
    return np.ascontiguousarray(out.astype(dtype))


# ----------------------------------------------------------------------------
# Bass programs.
# ----------------------------------------------------------------------------

def _build_conv(prep, layer2):
    """One GENConv layer over the gathered source window.

    layer2=False: out xout [bpc*128, H] bf16 (new node features x1).
    layer2=True : DeepGCN tail (LayerNorm+relu+residual) and per-core
    transposed column sums csout [128, 2*CHT] f32
    (csout[:, j] = colsum of channel tile j; first CHT cols x1, last CHT x2).
    """
    tu, bpc = prep["tu"], prep["bpc"]
    nc = bacc.Bacc("TRN2", target_bir_lowering=False, debug=False,
                   enable_asserts=False, num_devices=N_CORES)
    xg = nc.dram_tensor("xg", [tu * 128, H], BF16, kind="ExternalInput")
    ab = nc.dram_tensor("ab", [128, bpc * tu * 128], F8, kind="ExternalInput")
    wt = nc.dram_tensor("wt", [128, CHT * H], BF16, kind="ExternalInput")
    br = nc.dram_tensor("br", [128, H], F32, kind="ExternalInput")
    ts = nc.dram_tensor("ts", [128, 1], F32, kind="ExternalInput")
    if layer2:
        lngr = nc.dram_tensor("lngr", [128, H], F32, kind="ExternalInput")
        lnbr = nc.dram_tensor("lnbr", [128, H], F32, kind="ExternalInput")
        csout = nc.dram_tensor("csout", [128, 2 * CHT], F32, kind="ExternalOutput")
    else:
        xout = nc.dram_tensor("xout", [bpc * 128, H], F8, kind="ExternalOutput")
        xout_r = xout.rearrange("(n p) d -> n p d", p=128)

    xg_r = xg.rearrange("(n p) d -> n p d", p=128)

    with tile.TileContext(nc) as tc:
        with (
            tc.tile_pool(name="persist", bufs=1) as pp,
            tc.tile_pool(name="epi", bufs=2) as ep,
        ):
            # tiny params first (exp scale needed by the qw pass)
            ts_sb = pp.tile([128, 1], F32)
            nc.sync.dma_start(ts_sb[:], ts[:])
            eps_sb = pp.tile([128, 1], F32)
            nc.gpsimd.memset(eps_sb[:], EPS_MSG)
            # w is scaled by 1/64 inside the exp (softmax ratio is invariant)
            # so fp8e4m3 never overflows: exp(t*r) <= 448*64 is covered.
            lge_sb = pp.tile([128, 1], F32)
            nc.gpsimd.memset(lge_sb[:], float(-np.log(64.0)))

            # A slabs first: the first SpMM needs slab 0 + two qw tiles only
            ab_sb = pp.tile([128, bpc * tu * 128], F8)
            for bl in range(bpc):
                nc.sync.dma_start(ab_sb[:, bl * tu * 128:(bl + 1) * tu * 128],
                                  ab[:, bl * tu * 128:(bl + 1) * tu * 128])
            ab_r = ab_sb[:].rearrange("p (b t k m) -> p b t k m", b=bpc, k=2, m=128)

            # source pass: r = relu(x); w8 = exp(t*r); q8 = r*w8  (fp8 operands)
            xg_sb = pp.tile([128, tu * H], BF16)
            qw8 = pp.tile([128, tu * 2 * H], F8)
            qw8_r = qw8[:].rearrange("p (t f) -> p t f", f=2 * H)
            for s in range(tu):
                xs = xg_sb[:, s * H:(s + 1) * H]
                nc.sync.dma_start(xs, xg_r[s])
                q8 = qw8[:, 2 * s * H:(2 * s + 1) * H]
                w8 = qw8[:, (2 * s + 1) * H:(2 * s + 2) * H]
                # e = exp(t*x)/64; for t>0: w = exp(t*relu(x))/64 = max(e, 1/64)
                # and q = relu(x)*w = (x max 0)*e exactly (both fused DVE ops).
                e = ep.tile([128, H], BF16, tag="e")
                nc.scalar.activation(e[:], xs, AF.Exp,
                                     bias=lge_sb[:, 0:1], scale=ts_sb[:, 0:1])
                nc.vector.scalar_tensor_tensor(q8, xs, 0.0, e[:],
                                               ALU.max, ALU.mult)
                nc.vector.tensor_scalar_max(w8, e[:], 1.0 / 64.0)

            wt_sb = pp.tile([128, CHT * H], BF16)
            nc.sync.dma_start(wt_sb[:], wt[:])
            br_sb = pp.tile([128, H], F32)
            nc.sync.dma_start(br_sb[:], br[:])
            ident = pp.tile([128, 128], F32)
            masks.make_identity(nc, ident[:])
            if layer2:
                lng_sb = pp.tile([128, H], F32)
                lnb_sb = pp.tile([128, H], F32)
                nc.sync.dma_start(lng_sb[:], lngr[:])
                nc.sync.dma_start(lnb_sb[:], lnbr[:])
                lneps_sb = pp.tile([128, 1], F32)
                nc.gpsimd.memset(lneps_sb[:], LN_EPS)
                ones_sb = pp.tile([128, 1], BF16)
                nc.gpsimd.memset(ones_sb[:], 1.0)
                acc1 = pp.tile([128, H], F32)
                acc2 = pp.tile([128, H], F32)
                nc.gpsimd.memset(acc1[:], 0.0)
                nc.gpsimd.memset(acc2[:], 0.0)

            with (
                tc.tile_pool(name="psA", bufs=2, space="PSUM") as psA,
                tc.tile_pool(name="psE", bufs=2, space="PSUM") as psE,
            ):
                aggs = [None] * bpc

                def spmm(bl):
                    agg = psA.tile([128, 2 * H], F32, tag="agg")
                    aggs[bl] = agg
                    for td in range(tu // 2):
                        at2 = ab_r[:, bl, td]
                        for ch in range(3):
                            nc.tensor.matmul(
                                agg[:, ch * 512:(ch + 1) * 512],
                                at2,
                                qw8_r[:, 2 * td:2 * td + 2, ch * 512:(ch + 1) * 512],
                                start=(td == 0), stop=(td == tu // 2 - 1),
                                perf_mode=DR,
                            )

                def epilogue(bl):
                    agg = aggs[bl]
                    xo = xg_sb[:, bl * H:(bl + 1) * H]      # own x tile (bf16)
                    m = ep.tile([128, H], F32, tag="m")
                    rec = ep.tile([128, H], F32, tag="rec")
                    nc.vector.reciprocal_approx_fast(rec[:], agg[:, H:2 * H])
                    nc.vector.tensor_mul(m[:], agg[:, 0:H], rec[:])
                    nc.vector.tensor_add(m[:], m[:], xo)
                    # transpose m -> lhsT tiles, then x_new = m @ W.T + b
                    mt = ep.tile([128, H], BF16, tag="mt")
                    for c in range(CHT):
                        tp = psE.tile([128, 128], F32, tag="e")
                        nc.tensor.transpose(tp[:], m[:, c * 128:(c + 1) * 128], ident[:])
                        nc.scalar.copy(mt[:, c * 128:(c + 1) * 128], tp[:])
                    xps = psA.tile([128, H], F32, tag="agg")
                    for c in range(CHT):
                        nc.tensor.matmul(
                            xps[:, 0:512], mt[:, c * 128:(c + 1) * 128],
                            wt_sb[:, c * H:c * H + 512],
                            start=(c == 0), stop=(c == CHT - 1))
                        nc.tensor.matmul(
                            xps[:, 512:H], mt[:, c * 128:(c + 1) * 128],
                            wt_sb[:, c * H + 512:(c + 1) * H],
                            start=(c == 0), stop=(c == CHT - 1))
                    if not layer2:
                        xnb = ep.tile([128, H], F8, tag="xnb")
                        nc.vector.tensor_add(xnb[:], xps[:], br_sb[:])
                        nc.sync.dma_start(xout_r[bl], xnb[:])
                    else:
                        xn = ep.tile([128, H], F32, tag="xn")
                        nc.vector.tensor_add(xn[:], xps[:], br_sb[:])
                        # LayerNorm over channels, relu, then x2 = hn + x1_own
                        stats = ep.tile([128, 3, 6], F32, tag="bnst")
                        xn_r = xn[:].rearrange("p (g f) -> p g f", f=256)
                        for g3 in range(3):
                            nc.vector.bn_stats(stats[:, g3, :], xn_r[:, g3])
                        mv = ep.tile([128, 2], F32, tag="mv")
                        nc.vector.bn_aggr(mv[:], stats[:])
                        rstd = ep.tile([128, 1], F32, tag="rstd")
                        veps = ep.tile([128, 1], F32, tag="veps")
                        nc.vector.tensor_scalar(veps[:], mv[:, 1:2],
                                                lneps_sb[:, 0:1], None, ALU.add)
                        nc.vector.reciprocal_approx_fast(rstd[:], veps[:])
                        nc.scalar.sqrt(rstd[:], rstd[:])
                        nmr = ep.tile([128, 1], F32, tag="nmr")
                        nc.vector.tensor_mul(nmr[:], mv[:, 0:1], rstd[:])
                        nc.vector.tensor_scalar_mul(nmr[:], nmr[:], -1.0)
                        hn = ep.tile([128, H], F32, tag="hn")
                        nc.scalar.activation(hn[:], xn[:], AF.Identity,
                                             bias=nmr[:, 0:1], scale=rstd[:, 0:1])
                        nc.gpsimd.tensor_mul(hn[:], hn[:], lng_sb[:])
                        nc.gpsimd.tensor_add(hn[:], hn[:], lnb_sb[:])
                        nc.scalar.activation(hn[:], hn[:], AF.Relu)
                        x2 = ep.tile([128, H], F32, tag="x2")
                        nc.gpsimd.tensor_add(x2[:], hn[:], xo)
                        # per-partition running sums (cross-partition reduce later)
                        nc.gpsimd.tensor_add(acc1[:], acc1[:], xo)
                        nc.gpsimd.tensor_add(acc2[:], acc2[:], x2[:])

                # software pipeline: SpMM(bl+1) issues before epilogue(bl)
                spmm(0)
                for bl in range(1, bpc):
                    spmm(bl)
                    epilogue(bl - 1)
                epilogue(bpc - 1)

            if layer2:
                with tc.tile_pool(name="psF", bufs=1, space="PSUM") as psF:
                    # transposed column sums: cs[:, j] = colsum of channel tile j
                    acc1b = pp.tile([128, H], BF16)
                    acc2b = pp.tile([128, H], BF16)
                    nc.scalar.copy(acc1b[:], acc1[:])
                    nc.scalar.copy(acc2b[:], acc2[:])
                    cs_ps = psF.tile([128, 2 * CHT], F32)
                    for j in range(CHT):
                        nc.tensor.matmul(cs_ps[:, j:j + 1],
                                         acc1b[:, j * 128:(j + 1) * 128],
                                         ones_sb[:], start=True, stop=True)
                        nc.tensor.matmul(cs_ps[:, CHT + j:CHT + j + 1],
                                         acc2b[:, j * 128:(j + 1) * 128],
                                         ones_sb[:], start=True, stop=True)
                    cs_sb = pp.tile([128, 2 * CHT], F32)
                    nc.scalar.copy(cs_sb[:], cs_ps[:])
                    nc.sync.dma_start(csout[:], cs_sb[:])
    nc.compile()
    return nc


def _build_final(n):
    """Sum per-core transposed colsum partials, matvec through Wc, + bc + x0."""
    nc = bacc.Bacc("TRN2", target_bir_lowering=False, debug=False,
                   enable_asserts=False, num_devices=N_CORES)
    parts = nc.dram_tensor("parts", [128, N_CORES * 2 * CHT], F32, kind="ExternalInput")
    wct = nc.dram_tensor("wct", [128, 2 * CHT * H], BF16, kind="ExternalInput")
    bcr = nc.dram_tensor("bcr", [1, H], F32, kind="ExternalInput")
    x0r = nc.dram_tensor("x0r", [1, H], F32, kind="ExternalInput")
    row0 = nc.dram_tensor("row0", [1, H], F32, kind="ExternalOutput")

    with tile.TileContext(nc) as tc:
        with (
            tc.tile_pool(name="sb", bufs=1) as sb,
            tc.tile_pool(name="ps", bufs=1, space="PSUM") as ps,
        ):
            wct_sb = sb.tile([128, 2 * CHT * H], BF16)
            nc.sync.dma_start(wct_sb[:], wct[:])
            pt = sb.tile([128, N_CORES * 2 * CHT], F32)
            nc.sync.dma_start(pt[:], parts[:])
            acc = sb.tile([128, 2 * CHT], F32)
            nc.vector.tensor_reduce(
                acc[:], pt[:].rearrange("p (a d) -> p d a", a=N_CORES),
                mybir.AxisListType.X, ALU.add)
            nc.vector.tensor_scalar_mul(acc[:], acc[:], 1.0 / n)
            accb = sb.tile([128, 2 * CHT], BF16)
            nc.scalar.copy(accb[:], acc[:])

            g_ps = ps.tile([1, H], F32)
            for j in range(2 * CHT):
                for lo, hi in ((0, 512), (512, H)):   # per-bank chunks
                    nc.tensor.matmul(
                        g_ps[:, lo:hi],
                        accb[:, j:j + 1],
                        wct_sb[:, j * H + lo:j * H + hi],
                        start=(j == 0), stop=(j == 2 * CHT - 1))
            bc_sb = sb.tile([1, H], F32)
            x0_sb = sb.tile([1, H], F32)
            out_sb = sb.tile([1, H], F32)
            nc.sync.dma_start(bc_sb[:], bcr[:])
            nc.sync.dma_start(x0_sb[:], x0r[:])
            nc.vector.tensor_add(out_sb[:], g_ps[:], bc_sb[:])
            nc.vector.tensor_add(out_sb[:], out_sb[:], x0_sb[:])
            nc.sync.dma_start(row0[:], out_sb[:])
    nc.compile()
    return nc


def kernel(**inputs):
    x = np.asarray(inputs["x"], dtype=np.float32)
    w1 = np.asarray(inputs["W1"], dtype=np.float32)
    b1 = np.asarray(inputs["b1"], dtype=np.float32)
    t1 = np.float32(np.asarray(inputs["t1"]))
    w2 = np.asarray(inputs["W2"], dtype=np.float32)
    b2 = np.asarray(inputs["b2"], dtype=np.float32)
    t2 = np.float32(np.asarray(inputs["t2"]))
    ln_g = np.asarray(inputs["ln_g"], dtype=np.float32)
    ln_b = np.asarray(inputs["ln_b"], dtype=np.float32)
    wc = np.asarray(inputs["Wc"], dtype=np.float32)
    bc = np.asarray(inputs["bc"], dtype=np.float32)
    ei = np.asarray(inputs["edge_index"])

    n = x.shape[1]
    ekey = (ei.shape[1], n,
            int(np.bitwise_xor.reduce(ei[0].astype(np.int64) * 31 + ei[1])))
    if ekey not in _cache:
        prep = _prepare(ei, n)
        progs = dict(
            conv=_build_conv(prep, False),
            tail=_build_conv(prep, True),
            fin=_build_final(n),
        )
        _cache[ekey] = (prep, progs)
    prep, progs = _cache[ekey]

    xp = np.ascontiguousarray(x[0][prep["perm"]])    # permuted node features
    t1r = np.full((128, 1), t1, dtype=np.float32)
    t2r = np.full((128, 1), t2, dtype=np.float32)
    w1t, w2t = _pack_wt(w1), _pack_wt(w2)
    b1r = np.ascontiguousarray(np.broadcast_to(b1, (128, H)))
    b2r = np.ascontiguousarray(np.broadcast_to(b2, (128, H)))
    lngr = np.ascontiguousarray(np.broadcast_to(ln_g, (128, H)))
    lnbr = np.ascontiguousarray(np.broadcast_to(ln_b, (128, H)))

    cores = list(range(N_CORES))

    # --- launch 1: conv1 ---
    maps1 = [dict(xg=_gather_rows(xp, prep["glists"][c]), ab=prep["abands"][c],
                  wt=w1t, br=b1r, ts=t1r) for c in cores]
    res1 = run_bass_kernel_spmd(progs["conv"], maps1, core_ids=cores)
    x1 = np.concatenate([res1.results[c]["xout"] for c in cores], axis=0)

    # --- launch 2: conv2 + LN + transposed colsums ---
    maps2 = [dict(xg=_gather_rows(x1, prep["glists"][c]), ab=prep["abands"][c],
                  wt=w2t, br=b2r, ts=t2r, lngr=lngr, lnbr=lnbr) for c in cores]
    res2 = run_bass_kernel_spmd(progs["tail"], maps2, core_ids=cores)
    parts = np.ascontiguousarray(
        np.concatenate([res2.results[c]["csout"] for c in cores], axis=1))

    # --- launch 3: finalize row 0 ---
    maps3 = [dict(parts=parts, wct=_pack_wt(wc),
                  bcr=bc.reshape(1, H).astype(np.float32),
                  x0r=np.ascontiguousarray(x[0, 0:1, :])) for _ in cores]
    res3 = run_bass_kernel_spmd(progs["fin"], maps3, core_ids=cores)
    row0 = res3.results[0]["row0"][0]

    out = x.copy()
    out[0, 0, :] = row0
    return out


# revision 21
# speedup vs baseline: 1.1256x; 1.0816x over previous
"""Trainium2 Bass kernel for a 2-layer GENConv (softmax aggr) + LayerNorm GNN block.

Distribution: graph-partitioned across 8 NeuronCores via a Fiedler (spectral 1D)
node ordering. The per-channel softmax aggregation collapses to two SpMMs
because GENConv softmax logits depend only on the source node:

  r = relu(x); w = exp(t*r); q = w*r
  num = A @ q;  den = A @ w;  agg = num/den + eps     (exactly the reference
  softmax aggregation; the max-shift and the eps inside exp cancel)

Each core owns 4 contiguous dst blocks of 128 nodes and receives a GATHERED
source window: its 4 own tiles first, then the sorted distinct out-of-own
source rows (halo), zero-padded to a uniform TU tiles (SPMD uniformity lives
in the data, not the code). The A^T slabs are fp8 (0/1 counts, exact) and the
q/w operands are fp8, so the SpMM runs in DoubleRow (double-pumped) mode.
Block epilogues are software-pipelined behind the next block's SpMM so the
tensor engine never waits on the vector division chain.

Three SPMD launches: conv1 -> (host regathers x1) -> conv2 + LN + per-core
transposed column sums -> (host stacks partials) -> tiny finalize matvec.
The host does no float arithmetic: only slicing/permutation/concatenation.
"""

import ml_dtypes
import numpy as np

import concourse.bass as bass
import concourse.bacc as bacc
import concourse.mybir as mybir
import concourse.tile as tile
import concourse.masks as masks
from concourse.bass_utils import run_bass_kernel_spmd

F32 = mybir.dt.float32
BF16 = mybir.dt.bfloat16
F8 = mybir.dt.float8e4
AF = mybir.ActivationFunctionType
ALU = mybir.AluOpType
DR = mybir.MatmulPerfMode.DoubleRow

N_CORES = 8
H = 768
CHT = H // 128           # channel tiles = 6
EPS_MSG = 1e-7
LN_EPS = 1e-5

_cache = {}


# ----------------------------------------------------------------------------
# Host-side graph preprocessing (index work only — no float math on data).
# ----------------------------------------------------------------------------

def _ordering(src, dst, n):
    """1D spectral (Fiedler) layout of the graph; falls back to RCM/identity."""
    import scipy.sparse as sp
    a = sp.csr_matrix(
        (np.ones(len(src), dtype=np.float64), (dst, src)), shape=(n, n)
    )
    asym = ((a + a.T) > 0).astype(np.float64)
    try:
        from scipy.sparse.linalg import eigsh
        lap = sp.diags(np.asarray(asym.sum(1)).ravel()) - asym
        _, vecs = eigsh(lap, k=2, sigma=-1e-4, which="LM")
        return np.argsort(vecs[:, 1]).astype(np.int64)
    except Exception:
        try:
            from scipy.sparse.csgraph import reverse_cuthill_mckee
            return np.asarray(
                reverse_cuthill_mckee(asym.tocsr(), symmetric_mode=True)
            ).astype(np.int64)
        except Exception:
            return np.arange(n, dtype=np.int64)


def _prepare(edge_index, n):
    src = np.asarray(edge_index[0], dtype=np.int64)
    dst = np.asarray(edge_index[1], dtype=np.int64)
    perm = _ordering(src, dst, n)           # new position i holds old node perm[i]
    inv = np.empty(n, dtype=np.int64)
    inv[perm] = np.arange(n)
    ns, nd = inv[src], inv[dst]             # edges in new coordinates

    nb = n // 128
    bpc = nb // N_CORES                     # dst blocks per core (4)

    # per-core gathered source lists: [own rows | sorted halo rows | -1 pad]
    halos = []
    for c in range(N_CORES):
        lo, hi = c * bpc * 128, (c + 1) * bpc * 128
        m = (nd >= lo) & (nd < hi)
        srcs = np.unique(ns[m])
        halos.append(srcs[(srcs < lo) | (srcs >= hi)])
    tu = bpc + max((len(h) + 127) // 128 for h in halos)   # uniform tiles
    tu += tu % 2                                           # even (DoubleRow pairs)
    glists, abands = [], []
    for c in range(N_CORES):
        lo, hi = c * bpc * 128, (c + 1) * bpc * 128
        glist = np.full(tu * 128, -1, dtype=np.int64)
        nown = hi - lo
        glist[:nown] = np.arange(lo, hi)
        glist[nown:nown + len(halos[c])] = halos[c]
        glists.append(glist)
        pos = np.full(n, -1, dtype=np.int64)
        valid = glist >= 0
        pos[glist[valid]] = np.arange(tu * 128)[valid]
        m = (nd >= lo) & (nd < hi)
        p = pos[ns[m]]
        d = nd[m] - lo
        assert (p >= 0).all()
        ab = np.zeros((128, bpc * tu * 128), dtype=np.float32)
        np.add.at(ab, (p % 128, ((d // 128) * tu + p // 128) * 128 + d % 128), 1.0)
        abands.append(ab.astype(ml_dtypes.float8_e4m3fn))

    return dict(perm=perm, inv=inv, tu=tu, bpc=bpc, glists=glists,
                abands=abands)


def _gather_rows(full, glist):
    """full[glist] with -1 -> zero row; returns fp8e4m3."""
    out = np.zeros((len(glist), full.shape[1]), dtype=ml_dtypes.float8_e4m3fn)
    sel = glist >= 0
    out[sel] = full[glist[sel]].astype(ml_dtypes.float8_e4m3fn)
    return out


def _pack_wt(w, dtype=ml_dtypes.bfloat16):
    """[Hout, Hin] weight -> partition-major packed W.T tiles [128, (Hin/128)*Hout]:
    out[p, c*Hout + o] = W[o, c*128 + p]"""
    h_out, h_in = w.shape
    nt = h_in // 128
    out = np.empty((128, nt * h_out), dtype=np.float32)
    for c in range(nt):
        out[:, c * h_out:(c + 1) * h_out] = w[:, c * 128:(c + 1) * 128].T
    return np.ascontiguousarray(out.astype(dtype))


# ----------------------------------------------------------------------------
# Bass programs.
# ----------------------------------------------------------------------------

def _build_conv(prep, layer2):
    """One GENConv layer over the gathered source window.

    layer2=False: out xout [bpc*128, H] bf16 (new node features x1).
    layer2=True : DeepGCN tail (LayerNorm+relu+residual) and per-core
    transposed column sums csout [128, 2*CHT] f32
    (csout[:, j] = colsum of channel tile j; first CHT cols x1, last CHT x2).
    """
    tu, bpc = prep["tu"], prep["bpc"]
    nc = bacc.Bacc("TRN2", target_bir_lowering=False, debug=False,
                   enable_asserts=False, num_devices=N_CORES)
    xg = nc.dram_tensor("xg", [tu * 128, H], F8, kind="ExternalInput")
    ab = nc.dram_tensor("ab", [128, bpc * tu * 128], F8, kind="ExternalInput")
    wt = nc.dram_tensor("wt", [128, CHT * H], BF16, kind="ExternalInput")
    br = nc.dram_tensor("br", [128, H], F32, kind="ExternalInput")
    ts = nc.dram_tensor("ts", [128, 1], F32, kind="ExternalInput")
    if layer2:
        lngr = nc.dram_tensor("lngr", [128, H], F32, kind="ExternalInput")
        lnbr = nc.dram_tensor("lnbr", [128, H], F32, kind="ExternalInput")
        csout = nc.dram_tensor("csout", [128, 2 * CHT], F32, kind="ExternalOutput")
    else:
        xout = nc.dram_tensor("xout", [bpc * 128, H], F8, kind="ExternalOutput")
        xout_r = xout.rearrange("(n p) d -> n p d", p=128)

    xg_r = xg.rearrange("(n p) d -> n p d", p=128)

    with tile.TileContext(nc) as tc:
        with (
            tc.tile_pool(name="persist", bufs=1) as pp,
            tc.tile_pool(name="epi", bufs=2) as ep,
        ):
            # tiny params first (exp scale needed by the qw pass)
            ts_sb = pp.tile([128, 1], F32)
            nc.sync.dma_start(ts_sb[:], ts[:])
            eps_sb = pp.tile([128, 1], F32)
            nc.gpsimd.memset(eps_sb[:], EPS_MSG)
            # w is scaled by 1/64 inside the exp (softmax ratio is invariant)
            # so fp8e4m3 never overflows: exp(t*r) <= 448*64 is covered.
            lge_sb = pp.tile([128, 1], F32)
            nc.gpsimd.memset(lge_sb[:], float(-np.log(64.0)))

            # A slabs first: the first SpMM needs slab 0 + two qw tiles only
            ab_sb = pp.tile([128, bpc * tu * 128], F8)
            for bl in range(bpc):
                nc.sync.dma_start(ab_sb[:, bl * tu * 128:(bl + 1) * tu * 128],
                                  ab[:, bl * tu * 128:(bl + 1) * tu * 128])
            ab_r = ab_sb[:].rearrange("p (b t k m) -> p b t k m", b=bpc, k=2, m=128)

            # source pass: r = relu(x); w8 = exp(t*r); q8 = r*w8  (fp8 operands)
            xg_sb = pp.tile([128, tu * H], F8)
            qw8 = pp.tile([128, tu * 2 * H], F8)
            qw8_r = qw8[:].rearrange("p (t f) -> p t f", f=2 * H)
            for s in range(tu):
                xs = xg_sb[:, s * H:(s + 1) * H]
                nc.sync.dma_start(xs, xg_r[s])
                q8 = qw8[:, 2 * s * H:(2 * s + 1) * H]
                w8 = qw8[:, (2 * s + 1) * H:(2 * s + 2) * H]
                rs = ep.tile([128, H], BF16, tag="rs")
                ws = ep.tile([128, H], BF16, tag="ws")
                nc.vector.tensor_scalar_max(rs[:], xs, 0.0)
                nc.scalar.activation(ws[:], rs[:], AF.Exp,
                                     bias=lge_sb[:, 0:1], scale=ts_sb[:, 0:1])
                nc.vector.tensor_mul(q8, rs[:], ws[:])
                nc.vector.tensor_scalar_mul(w8, ws[:], 1.0)

            wt_sb = pp.tile([128, CHT * H], BF16)
            nc.sync.dma_start(wt_sb[:], wt[:])
            br_sb = pp.tile([128, H], F32)
            nc.sync.dma_start(br_sb[:], br[:])
            ident = pp.tile([128, 128], F32)
            masks.make_identity(nc, ident[:])
            if layer2:
                lng_sb = pp.tile([128, H], F32)
                lnb_sb = pp.tile([128, H], F32)
                nc.sync.dma_start(lng_sb[:], lngr[:])
                nc.sync.dma_start(lnb_sb[:], lnbr[:])
                lneps_sb = pp.tile([128, 1], F32)
                nc.gpsimd.memset(lneps_sb[:], LN_EPS)
                ones_sb = pp.tile([128, 1], BF16)
                nc.gpsimd.memset(ones_sb[:], 1.0)
                acc1 = pp.tile([128, H], F32)
                acc2 = pp.tile([128, H], F32)
                nc.gpsimd.memset(acc1[:], 0.0)
                nc.gpsimd.memset(acc2[:], 0.0)

            with (
                tc.tile_pool(name="psA", bufs=2, space="PSUM") as psA,
                tc.tile_pool(name="psE", bufs=2, space="PSUM") as psE,
            ):
                aggs = [None] * bpc

                def spmm(bl):
                    agg = psA.tile([128, 2 * H], F32, tag="agg")
                    aggs[bl] = agg
                    for td in range(tu // 2):
                        at2 = ab_r[:, bl, td]
                        for ch in range(3):
                            nc.tensor.matmul(
                                agg[:, ch * 512:(ch + 1) * 512],
                                at2,
                                qw8_r[:, 2 * td:2 * td + 2, ch * 512:(ch + 1) * 512],
                                start=(td == 0), stop=(td == tu // 2 - 1),
                                perf_mode=DR,
                            )

                def epilogue(bl):
                    agg = aggs[bl]
                    xo = xg_sb[:, bl * H:(bl + 1) * H]      # own x tile (bf16)
                    m = ep.tile([128, H], F32, tag="m")
                    rec = ep.tile([128, H], F32, tag="rec")
                    nc.vector.reciprocal_approx_fast(rec[:], agg[:, H:2 * H])
                    nc.vector.tensor_mul(m[:], agg[:, 0:H], rec[:])
                    nc.vector.tensor_add(m[:], m[:], xo)
                    # transpose m -> lhsT tiles, then x_new = m @ W.T + b
                    mt = ep.tile([128, H], BF16, tag="mt")
                    for c in range(CHT):
                        tp = psE.tile([128, 128], F32, tag="e")
                        nc.tensor.transpose(tp[:], m[:, c * 128:(c + 1) * 128], ident[:])
                        nc.scalar.copy(mt[:, c * 128:(c + 1) * 128], tp[:])
                    xps = psA.tile([128, H], F32, tag="agg")
                    for c in range(CHT):
                        nc.tensor.matmul(
                            xps[:, 0:512], mt[:, c * 128:(c + 1) * 128],
                            wt_sb[:, c * H:c * H + 512],
                            start=(c == 0), stop=(c == CHT - 1))
                        nc.tensor.matmul(
                            xps[:, 512:H], mt[:, c * 128:(c + 1) * 128],
                            wt_sb[:, c * H + 512:(c + 1) * H],
                            start=(c == 0), stop=(c == CHT - 1))
                    if not layer2:
                        xnb = ep.tile([128, H], F8, tag="xnb")
                        nc.vector.tensor_add(xnb[:], xps[:], br_sb[:])
                        nc.sync.dma_start(xout_r[bl], xnb[:])
                    else:
                        xn = ep.tile([128, H], F32, tag="xn")
                        nc.vector.tensor_add(xn[:], xps[:], br_sb[:])
                        # LayerNorm over channels, relu, then x2 = hn + x1_own
                        stats = ep.tile([128, 3, 6], F32, tag="bnst")
                        xn_r = xn[:].rearrange("p (g f) -> p g f", f=256)
                        for g3 in range(3):
                            nc.vector.bn_stats(stats[:, g3, :], xn_r[:, g3])
                        mv = ep.tile([128, 2], F32, tag="mv")
                        nc.vector.bn_aggr(mv[:], stats[:])
                        rstd = ep.tile([128, 1], F32, tag="rstd")
                        veps = ep.tile([128, 1], F32, tag="veps")
                        nc.vector.tensor_scalar(veps[:], mv[:, 1:2],
                                                lneps_sb[:, 0:1], None, ALU.add)
                        nc.vector.reciprocal_approx_fast(rstd[:], veps[:])
                        nc.scalar.sqrt(rstd[:], rstd[:])
                        nmr = ep.tile([128, 1], F32, tag="nmr")
                        nc.vector.tensor_mul(nmr[:], mv[:, 0:1], rstd[:])
                        nc.vector.tensor_scalar_mul(nmr[:], nmr[:], -1.0)
                        hn = ep.tile([128, H], F32, tag="hn")
                        nc.scalar.activation(hn[:], xn[:], AF.Identity,
                                             bias=nmr[:, 0:1], scale=rstd[:, 0:1])
                        nc.gpsimd.tensor_mul(hn[:], hn[:], lng_sb[:])
                        nc.gpsimd.tensor_add(hn[:], hn[:], lnb_sb[:])
                        nc.scalar.activation(hn[:], hn[:], AF.Relu)
                        x2 = ep.tile([128, H], F32, tag="x2")
                        nc.gpsimd.tensor_add(x2[:], hn[:], xo)
                        # per-partition running sums (cross-partition reduce later)
                        nc.gpsimd.tensor_add(acc1[:], acc1[:], xo)
                        nc.gpsimd.tensor_add(acc2[:], acc2[:], x2[:])

                # software pipeline: SpMM(bl+1) issues before epilogue(bl)
                spmm(0)
                for bl in range(1, bpc):
                    spmm(bl)
                    epilogue(bl - 1)
                epilogue(bpc - 1)

            if layer2:
                with tc.tile_pool(name="psF", bufs=1, space="PSUM") as psF:
                    # transposed column sums: cs[:, j] = colsum of channel tile j
                    acc1b = pp.tile([128, H], BF16)
                    acc2b = pp.tile([128, H], BF16)
                    nc.scalar.copy(acc1b[:], acc1[:])
                    nc.scalar.copy(acc2b[:], acc2[:])
                    cs_ps = psF.tile([128, 2 * CHT], F32)
                    for j in range(CHT):
                        nc.tensor.matmul(cs_ps[:, j:j + 1],
                                         acc1b[:, j * 128:(j + 1) * 128],
                                         ones_sb[:], start=True, stop=True)
                        nc.tensor.matmul(cs_ps[:, CHT + j:CHT + j + 1],
                                         acc2b[:, j * 128:(j + 1) * 128],
                                         ones_sb[:], start=True, stop=True)
                    cs_sb = pp.tile([128, 2 * CHT], F32)
                    nc.scalar.copy(cs_sb[:], cs_ps[:])
                    nc.sync.dma_start(csout[:], cs_sb[:])
    nc.compile()
    return nc


def _build_final(n):
    """Sum per-core transposed colsum partials, matvec through Wc, + bc + x0."""
    nc = bacc.Bacc("TRN2", target_bir_lowering=False, debug=False,
                   enable_asserts=False, num_devices=N_CORES)
    parts = nc.dram_tensor("parts", [128, N_CORES * 2 * CHT], F32, kind="ExternalInput")
    wct = nc.dram_tensor("wct", [128, 2 * CHT * H], BF16, kind="ExternalInput")
    bcr = nc.dram_tensor("bcr", [1, H], F32, kind="ExternalInput")
    x0r = nc.dram_tensor("x0r", [1, H], F32, kind="ExternalInput")
    row0 = nc.dram_tensor("row0", [1, H], F32, kind="ExternalOutput")

    with tile.TileContext(nc) as tc:
        with (
            tc.tile_pool(name="sb", bufs=1) as sb,
            tc.tile_pool(name="ps", bufs=1, space="PSUM") as ps,
        ):
            wct_sb = sb.tile([128, 2 * CHT * H], BF16)
            nc.sync.dma_start(wct_sb[:], wct[:])
            pt = sb.tile([128, N_CORES * 2 * CHT], F32)
            nc.sync.dma_start(pt[:], parts[:])
            acc = sb.tile([128, 2 * CHT], F32)
            nc.vector.tensor_reduce(
                acc[:], pt[:].rearrange("p (a d) -> p d a", a=N_CORES),
                mybir.AxisListType.X, ALU.add)
            nc.vector.tensor_scalar_mul(acc[:], acc[:], 1.0 / n)
            accb = sb.tile([128, 2 * CHT], BF16)
            nc.scalar.copy(accb[:], acc[:])

            g_ps = ps.tile([1, H], F32)
            for j in range(2 * CHT):
                for lo, hi in ((0, 512), (512, H)):   # per-bank chunks
                    nc.tensor.matmul(
                        g_ps[:, lo:hi],
                        accb[:, j:j + 1],
                        wct_sb[:, j * H + lo:j * H + hi],
                        start=(j == 0), stop=(j == 2 * CHT - 1))
            bc_sb = sb.tile([1, H], F32)
            x0_sb = sb.tile([1, H], F32)
            out_sb = sb.tile([1, H], F32)
            nc.sync.dma_start(bc_sb[:], bcr[:])
            nc.sync.dma_start(x0_sb[:], x0r[:])
            nc.vector.tensor_add(out_sb[:], g_ps[:], bc_sb[:])
            nc.vector.tensor_add(out_sb[:], out_sb[:], x0_sb[:])
            nc.sync.dma_start(row0[:], out_sb[:])
    nc.compile()
    return nc


def kernel(**inputs):
    x = np.asarray(inputs["x"], dtype=np.float32)
    w1 = np.asarray(inputs["W1"], dtype=np.float32)
    b1 = np.asarray(inputs["b1"], dtype=np.float32)
    t1 = np.float32(np.asarray(inputs["t1"]))
    w2 = np.asarray(inputs["W2"], dtype=np.float32)
    b2 = np.asarray(inputs["b2"], dtype=np.float32)
    t2 = np.float32(np.asarray(inputs["t2"]))
    ln_g = np.asarray(inputs["ln_g"], dtype=np.float32)
    ln_b = np.asarray(inputs["ln_b"], dtype=np.float32)
    wc = np.asarray(inputs["Wc"], dtype=np.float32)
    bc = np.asarray(inputs["bc"], dtype=np.float32)
    ei = np.asarray(inputs["edge_index"])

    n = x.shape[1]
    ekey = (ei.shape[1], n,
            int(np.bitwise_xor.reduce(ei[0].astype(np.int64) * 31 + ei[1])))
    if ekey not in _cache:
        prep = _prepare(ei, n)
        progs = dict(
            conv=_build_conv(prep, False),
            tail=_build_conv(prep, True),
            fin=_build_final(n),
        )
        _cache[ekey] = (prep, progs)
    prep, progs = _cache[ekey]

    xp = np.ascontiguousarray(x[0][prep["perm"]])    # permuted node features
    t1r = np.full((128, 1), t1, dtype=np.float32)
    t2r = np.full((128, 1), t2, dtype=np.float32)
    w1t, w2t = _pack_wt(w1), _pack_wt(w2)
    b1r = np.ascontiguousarray(np.broadcast_to(b1, (128, H)))
    b2r = np.ascontiguousarray(np.broadcast_to(b2, (128, H)))
    lngr = np.ascontiguousarray(np.broadcast_to(ln_g, (128, H)))
    lnbr = np.ascontiguousarray(np.broadcast_to(ln_b, (128, H)))

    cores = list(range(N_CORES))

    # --- launch 1: conv1 ---
    maps1 = [dict(xg=_gather_rows(xp, prep["glists"][c]), ab=prep["abands"][c],
                  wt=w1t, br=b1r, ts=t1r) for c in cores]
    res1 = run_bass_kernel_spmd(progs["conv"], maps1, core_ids=cores)
    x1 = np.concatenate([res1.results[c]["xout"] for c in cores], axis=0)

    # --- launch 2: conv2 + LN + transposed colsums ---
    maps2 = [dict(xg=_gather_rows(x1, prep["glists"][c]), ab=prep["abands"][c],
                  wt=w2t, br=b2r, ts=t2r, lngr=lngr, lnbr=lnbr) for c in cores]
    res2 = run_bass_kernel_spmd(progs["tail"], maps2, core_ids=cores)
    parts = np.ascontiguousarray(
        np.concatenate([res2.results[c]["csout"] for c in cores], axis=1))

    # --- launch 3: finalize row 0 ---
    maps3 = [dict(parts=parts, wct=_pack_wt(wc),
                  bcr=bc.reshape(1, H).astype(np.float32),
                  x0r=np.ascontiguousarray(x[0, 0:1, :])) for _ in cores]
    res3 = run_bass_kernel_spmd(progs["fin"], maps3, core_ids=cores)
    row0 = res3.results[0]["row0"][0]

    out = x.copy()
    out[0, 0, :] = row0
    return out
